# revision 12
# baseline (speedup 1.0000x reference)
"""MoE top-2 routing kernel for 8 Trainium2 NeuronCores.

Strategy (expert-parallel with I-sharding, per spec sharding hint):
  - Host computes the (cheap) gate: softmax -> top-2 -> renormalized scores.
  - Each expert's MLP is split into SPLIT=4 shards along the intermediate
    dim I; the 8 experts x 4 shards = 32 shard-tasks are placed on an
    8-core x 4-slot grid (2 experts per slot column, paired big/small by
    routed token count) so per-core work is near the 874us PE ideal.
  - Weights are bf16 and fully SBUF-resident (128 KiB/partition for both
    layers), so tokens stream while weights load once: DMA drops from
    ~320 MB/core (fp32r re-streaming baseline) to ~120 MB/core and the
    kernel is PE-bound at ~1 cycle/row bf16.
  - Host sums the 4 I-shard partials per expert, applies combine scores
    + b2, and scatter-adds into the full output.

Per-core device work (C ~= 16.4k shard-token columns, 128 PE cycles each):
  fc1 quarter: h = gelu(x @ w1q + b1q)   8x8 mm chunks per 512-col tile
  fc2 quarter: y_partial = h @ w2q       8x8 mm chunks per 512-col tile
  PE floor ~= C * 128 cycles; with all 8 cores busy the chip power-throttles
  the PE to ~2.0 GHz (measured 254-263 ns per 512-row mm vs 213 ideal;
  1 core alone measures 219), so the envelope is ~1.05-1.1 ms/core.
  Measured 1.10 ms -> ~99% of the 8-core-busy PE envelope; DMA ~= 120 MB
  fully overlapped. fp8 DoubleRow (2x PE) is blocked by accuracy: e4m3
  rounding ~5% rms vs the 2% tolerance.
"""

import sys

sys.path.insert(0, "/opt/trn_rl_repo")

from contextlib import ExitStack

import numpy as np
import ml_dtypes

from concourse import bacc, mybir, tile
from concourse.bass_utils import run_bass_kernel_spmd

E, H, I = 8, 1024, 4096
TOP_K = 2
N_CORES = 8

SPLIT = 4          # I-shards per expert
NSLOT = 4          # shard slots per core (E * SPLIT / N_CORES)
IQ = I // SPLIT    # 1024 intermediate dims per shard
MC1 = IQ // 128    # fc1 output chunks per shard
KC1 = H // 128     # fc1 contraction chunks
MC2 = H // 128     # fc2 output chunks
KC2 = IQ // 128    # fc2 contraction chunks per shard

TT = 512           # token tile (one PSUM bank of fp32)

F32 = mybir.dt.float32
BF16 = mybir.dt.bfloat16
BF16_NP = ml_dtypes.bfloat16


def _token_tiles(caps):
    """Static schedule: (slot, col_offset, len) covering sum(caps) columns."""
    out = []
    off = 0
    for s, cap in enumerate(caps):
        o = 0
        while o < cap:
            ln = min(TT, cap - o)
            out.append((s, off + o, ln))
            o += ln
        off += cap
    return out


def _build_nc(caps, repeat=1, dummy_out=False):
    """One SPMD program: NSLOT resident expert-shard MLPs over token columns.

    dummy_out=True keeps the full compute + DMA volume but writes every
    tile's y to one small DRAM region, so timing runs don't churn a 67MB
    donated output buffer between calls (the axon tunnel time is bimodal
    under large-buffer churn, which poisons the repeat-delta estimate).
    """
    C = sum(caps)
    nc = bacc.Bacc(
        "TRN2", target_bir_lowering=False, debug=False, num_devices=N_CORES
    )
    xT = nc.dram_tensor("xT", [128, KC1, C], BF16, kind="ExternalInput").ap()
    w1p = nc.dram_tensor(
        "w1p", [NSLOT, 128, MC1, H], BF16, kind="ExternalInput"
    ).ap()
    w2p = nc.dram_tensor(
        "w2p", [NSLOT, 128, MC2, IQ], BF16, kind="ExternalInput"
    ).ap()
    b1p = nc.dram_tensor("b1p", [NSLOT, 128, MC1], F32, kind="ExternalInput").ap()
    yC = TT if dummy_out else C
    yT = nc.dram_tensor("yT", [128, MC2, yC], F32, kind="ExternalOutput").ap()

    gelu = mybir.ActivationFunctionType.Gelu

    with tile.TileContext(nc) as tc, ExitStack() as ctx:
        wpool = ctx.enter_context(tc.tile_pool(name="w", bufs=1))
        xpool = ctx.enter_context(tc.tile_pool(name="x", bufs=2))
        hpool = ctx.enter_context(tc.tile_pool(name="h", bufs=2))
        ypool = ctx.enter_context(tc.tile_pool(name="y", bufs=1))
        ps1 = ctx.enter_context(tc.tile_pool(name="ps1", bufs=4, space="PSUM"))
        ps2 = ctx.enter_context(tc.tile_pool(name="ps2", bufs=4, space="PSUM"))

        b1t = wpool.tile([128, NSLOT * MC1], F32)
        # Per-slot weight tiles: Tile tracks dependencies per tile, so slot
        # 0's first matmul waits only on slot 0's 2MB DMA (~7us) instead of
        # the whole 16MB preload (~45us single-launch startup bubble).
        w1s = [
            wpool.tile([128, MC1, H], BF16, name=f"w1s{s}", tag=f"w1s{s}")
            for s in range(NSLOT)
        ]
        w2s = [
            wpool.tile([128, MC2, IQ], BF16, name=f"w2s{s}", tag=f"w2s{s}")
            for s in range(NSLOT)
        ]
        nc.sync.dma_start(out=b1t[:], in_=b1p[:, :, :])
        for s in range(NSLOT):
            nc.gpsimd.dma_start(out=w1s[s][:], in_=w1p[s])
            nc.gpsimd.dma_start(out=w2s[s][:], in_=w2p[s])

        rep_ctx = tc.For_i(0, repeat, 1) if repeat > 1 else None
        if rep_ctx is not None:
            ctx.enter_context(rep_ctx)

        for s, t0, ln in _token_tiles(caps):
            xt = xpool.tile([128, KC1, ln], BF16, tag="xt")
            nc.sync.dma_start(out=xt[:], in_=xT[:, :, t0 : t0 + ln])

            # ht split in two half-tiles: Tile deps are per-tile, so fc2's
            # first matmul would otherwise wait for ALL 8 fc1 gelu drains
            # (~700ns PE bubble per tile). With halves, fc2's ic=0..3 chain
            # only waits on the first half, ready 4 groups earlier.
            HH = KC2 // 2
            hta = hpool.tile([128, HH, ln], BF16, tag="hta")
            htb = hpool.tile([128, KC2 - HH, ln], BF16, tag="htb")
            for mc in range(MC1):
                ps = ps1.tile([128, ln], F32, tag="ps1")
                for kc in range(KC1):
                    nc.tensor.matmul(
                        ps[:],
                        lhsT=w1s[s][:, mc, kc * 128 : (kc + 1) * 128],
                        rhs=xt[:, kc, :],
                        start=(kc == 0),
                        stop=(kc == KC1 - 1),
                    )
                hdst = hta[:, mc, :] if mc < HH else htb[:, mc - HH, :]
                nc.scalar.activation(
                    out=hdst,
                    in_=ps[:],
                    func=gelu,
                    bias=b1t[:, s * MC1 + mc : s * MC1 + mc + 1],
                    scale=1.0,
                )

            yt = ypool.tile([128, MC2, ln], F32, tag="yt")
            for mc in range(MC2):
                ps = ps2.tile([128, ln], F32, tag="ps2")
                for ic in range(KC2):
                    hsrc = hta[:, ic, :] if ic < HH else htb[:, ic - HH, :]
                    nc.tensor.matmul(
                        ps[:],
                        lhsT=w2s[s][:, mc, ic * 128 : (ic + 1) * 128],
                        rhs=hsrc,
                        start=(ic == 0),
                        stop=(ic == KC2 - 1),
                    )
                nc.vector.tensor_copy(out=yt[:, mc, :], in_=ps[:])
            if dummy_out:
                nc.gpsimd.dma_start(out=yT[:, :, :ln], in_=yt[:])
            else:
                nc.gpsimd.dma_start(out=yT[:, :, t0 : t0 + ln], in_=yt[:])

    nc.compile()
    return nc


def _route(x_flat, gate_w, gate_b, alpha):
    """Host gate: returns per-expert (row_indices, combine_scores)."""
    logits = x_flat @ gate_w + gate_b
    m = logits.max(axis=-1, keepdims=True)
    p = np.exp(logits - m)
    p /= p.sum(axis=-1, keepdims=True)
    idx = np.argpartition(p, E - TOP_K, axis=-1)[:, -TOP_K:]  # top-2 (unordered)
    vals = np.take_along_axis(p, idx, axis=-1)
    sc = vals / vals.sum(axis=-1, keepdims=True)
    sc = sc * alpha[idx]
    routes = []
    for e in range(E):
        mask = idx == e  # at most one True per row (top-k distinct)
        rows = np.nonzero(mask.any(axis=1))[0]
        scores = sc[mask]  # row-major order matches `rows`
        routes.append((rows, scores.astype(np.float32)))
    return routes


def _plan(counts):
    """Place 8 experts x SPLIT shards on the 8-core x NSLOT-slot grid.

    Slot s holds experts order[2s] (its SPLIT shards on cores 0..SPLIT-1)
    and order[2s+1] (on cores SPLIT..2*SPLIT-1); cap_s = max of the two
    counts. Sorted placement minimizes sum_s cap_s.
    """
    order = np.argsort(-np.asarray(counts), kind="stable")
    assign = [[None] * NSLOT for _ in range(N_CORES)]
    caps = []
    for s in range(NSLOT):
        ea, eb = int(order[2 * s]), int(order[2 * s + 1])
        for q in range(SPLIT):
            assign[q][s] = (ea, q)
            assign[SPLIT + q][s] = (eb, q)
        caps.append(int(max(counts[ea], counts[eb])))
    return assign, caps


def prepare(hidden_states, gate_w, gate_b, w1, b1, w2, b2, alpha):
    """Host routing + input prep. Returns (nc, in_maps, state)."""
    x = np.asarray(hidden_states, dtype=np.float32)
    gate_w = np.asarray(gate_w, dtype=np.float32)
    gate_b = np.asarray(gate_b, dtype=np.float32)
    w1 = np.asarray(w1, dtype=np.float32)
    b1 = np.asarray(b1, dtype=np.float32)
    w2 = np.asarray(w2, dtype=np.float32)
    b2 = np.asarray(b2, dtype=np.float32)
    alpha = np.asarray(alpha, dtype=np.float32)

    B, S, Hd = x.shape
    T = B * S
    xf = x.reshape(T, Hd)

    routes = _route(xf, gate_w, gate_b, alpha)
    counts = [len(r) for r, _ in routes]
    assign, caps = _plan(counts)
    C = sum(caps)
    offs = np.concatenate([[0], np.cumsum(caps)]).astype(int)

    nc = _build_nc(caps)

    # Per-expert packed tokens [128, KC1, cnt] bf16, shared by the SPLIT
    # cores that hold the expert's shards.
    xTe = {}
    for e in range(E):
        rows, _ = routes[e]
        xTe[e] = np.ascontiguousarray(
            xf[rows].astype(BF16_NP).T.reshape(KC1, 128, len(rows))
            .transpose(1, 0, 2)
        )

    def pack_w(wq):
        # [128k, kc-or-ic chunks * 128 m] from [K, M]: out[p, mc, kc*128+m]
        K, M = wq.shape
        return np.ascontiguousarray(
            wq.reshape(K // 128, 128, M // 128, 128)
            .transpose(1, 2, 0, 3)
            .reshape(128, M // 128, K)
            .astype(BF16_NP)
        )

    in_maps = []
    for c in range(N_CORES):
        xTc = np.zeros((128, KC1, C), dtype=BF16_NP)
        w1c = np.zeros((NSLOT, 128, MC1, H), dtype=BF16_NP)
        w2c = np.zeros((NSLOT, 128, MC2, IQ), dtype=BF16_NP)
        b1c = np.zeros((NSLOT, 128, MC1), dtype=np.float32)
        for s in range(NSLOT):
            e, q = assign[c][s]
            cnt = counts[e]
            xTc[:, :, offs[s] : offs[s] + cnt] = xTe[e]
            w1c[s] = pack_w(w1[e][:, q * IQ : (q + 1) * IQ])
            w2c[s] = pack_w(w2[e][q * IQ : (q + 1) * IQ, :])
            b1c[s] = b1[e][q * IQ : (q + 1) * IQ].reshape(MC1, 128).T
        in_maps.append({"xT": xTc, "w1p": w1c, "w2p": w2c, "b1p": b1c})

    state = dict(
        routes=routes, counts=counts, assign=assign, caps=caps, offs=offs,
        C=C, b2=b2, B=B, S=S, Hd=Hd, T=T,
    )
    return nc, in_maps, state


def finalize(results, state):
    routes, counts = state["routes"], state["counts"]
    assign, offs = state["assign"], state["offs"]
    b2 = state["b2"]
    T, Hd = state["T"], state["Hd"]
    C = state["C"]

    # Sum the SPLIT I-shard partials per expert: [Hd, cnt] each.
    ysum = {}
    for c in range(N_CORES):
        yTc = results[c]["yT"].transpose(1, 0, 2).reshape(Hd, C)
        for s in range(NSLOT):
            e, _q = assign[c][s]
            part = yTc[:, offs[s] : offs[s] + counts[e]]
            ysum[e] = part if e not in ysum else ysum[e] + part

    out = np.zeros((T, Hd), dtype=np.float32)
    for e in range(E):
        rows, scores = routes[e]
        if not len(rows):
            continue
        out[rows] += scores[:, None] * (ysum[e].T + b2[e])
    return out.reshape(state["B"], state["S"], Hd)


def kernel(hidden_states, gate_w, gate_b, w1, b1, w2, b2, alpha):
    nc, in_maps, state = prepare(
        hidden_states, gate_w, gate_b, w1, b1, w2, b2, alpha
    )
    res = run_bass_kernel_spmd(nc, in_maps, list(range(N_CORES)))
    return finalize(res.results, state)


# revision 13
# speedup vs baseline: 1.0157x; 1.0157x over previous
"""MoE top-2 routing kernel for 8 Trainium2 NeuronCores.

Strategy (expert-parallel with I-sharding, per spec sharding hint):
  - Host computes the (cheap) gate: softmax -> top-2 -> renormalized scores.
  - Each expert's MLP is split into SPLIT=4 shards along the intermediate
    dim I; the 8 experts x 4 shards = 32 shard-tasks are placed on an
    8-core x 4-slot grid (2 experts per slot column, paired big/small by
    routed token count) so per-core work is near the 874us PE ideal.
  - Weights are bf16 and fully SBUF-resident (128 KiB/partition for both
    layers), so tokens stream while weights load once: DMA drops from
    ~320 MB/core (fp32r re-streaming baseline) to ~120 MB/core and the
    kernel is PE-bound at ~1 cycle/row bf16.
  - Host sums the 4 I-shard partials per expert, applies combine scores
    + b2, and scatter-adds into the full output.

Per-core device work (C ~= 16.4k shard-token columns, 128 PE cycles each):
  fc1 quarter: h = gelu(x @ w1q + b1q)   8x8 mm chunks per 512-col tile
  fc2 quarter: y_partial = h @ w2q       8x8 mm chunks per 512-col tile
  PE floor ~= C * 128 cycles; with all 8 cores busy the chip power-throttles
  the PE to ~2.0 GHz (measured 254-263 ns per 512-row mm vs 213 ideal;
  1 core alone measures 219), so the envelope is ~1.05-1.1 ms/core.
  Measured 1.10 ms -> ~99% of the 8-core-busy PE envelope; DMA ~= 120 MB
  fully overlapped. fp8 DoubleRow (2x PE) is blocked by accuracy: e4m3
  rounding ~5% rms vs the 2% tolerance.
"""

import sys

sys.path.insert(0, "/opt/trn_rl_repo")

from contextlib import ExitStack

import numpy as np
import ml_dtypes

from concourse import bacc, mybir, tile
from concourse.bass_utils import run_bass_kernel_spmd

E, H, I = 8, 1024, 4096
TOP_K = 2
N_CORES = 8

SPLIT = 4          # I-shards per expert
NSLOT = 4          # shard slots per core (E * SPLIT / N_CORES)
IQ = I // SPLIT    # 1024 intermediate dims per shard
MC1 = IQ // 128    # fc1 output chunks per shard
KC1 = H // 128     # fc1 contraction chunks
MC2 = H // 128     # fc2 output chunks
KC2 = IQ // 128    # fc2 contraction chunks per shard

TT = 512           # token tile (one PSUM bank of fp32)

F32 = mybir.dt.float32
BF16 = mybir.dt.bfloat16
BF16_NP = ml_dtypes.bfloat16


def _token_tiles(caps):
    """Static schedule: (slot, col_offset, len) covering sum(caps) columns."""
    out = []
    off = 0
    for s, cap in enumerate(caps):
        o = 0
        while o < cap:
            ln = min(TT, cap - o)
            out.append((s, off + o, ln))
            o += ln
        off += cap
    return out


def _build_nc(caps, repeat=1, dummy_out=False):
    """One SPMD program: NSLOT resident expert-shard MLPs over token columns.

    dummy_out=True keeps the full compute + DMA volume but writes every
    tile's y to one small DRAM region, so timing runs don't churn a 67MB
    donated output buffer between calls (the axon tunnel time is bimodal
    under large-buffer churn, which poisons the repeat-delta estimate).
    """
    C = sum(caps)
    nc = bacc.Bacc(
        "TRN2", target_bir_lowering=False, debug=False, num_devices=N_CORES
    )
    xT = nc.dram_tensor("xT", [128, KC1, C], BF16, kind="ExternalInput").ap()
    w1p = nc.dram_tensor(
        "w1p", [NSLOT, 128, MC1, H], BF16, kind="ExternalInput"
    ).ap()
    w2p = nc.dram_tensor(
        "w2p", [NSLOT, 128, MC2, IQ], BF16, kind="ExternalInput"
    ).ap()
    b1p = nc.dram_tensor("b1p", [NSLOT, 128, MC1], F32, kind="ExternalInput").ap()
    yC = TT if dummy_out else C
    yT = nc.dram_tensor("yT", [128, MC2, yC], F32, kind="ExternalOutput").ap()

    gelu = mybir.ActivationFunctionType.Gelu

    with tile.TileContext(nc) as tc, ExitStack() as ctx:
        wpool = ctx.enter_context(tc.tile_pool(name="w", bufs=1))
        xpool = ctx.enter_context(tc.tile_pool(name="x", bufs=2))
        hpool = ctx.enter_context(tc.tile_pool(name="h", bufs=2))
        ypool = ctx.enter_context(tc.tile_pool(name="y", bufs=1))
        ps1 = ctx.enter_context(tc.tile_pool(name="ps1", bufs=3, space="PSUM"))
        ps2 = ctx.enter_context(tc.tile_pool(name="ps2", bufs=3, space="PSUM"))

        b1t = wpool.tile([128, NSLOT * MC1], F32)
        # Per-slot weight tiles: Tile tracks dependencies per tile, so slot
        # 0's first matmul waits only on slot 0's 2MB DMA (~7us) instead of
        # the whole 16MB preload (~45us single-launch startup bubble).
        w1s = [
            wpool.tile([128, MC1, H], BF16, name=f"w1s{s}", tag=f"w1s{s}")
            for s in range(NSLOT)
        ]
        w2s = [
            wpool.tile([128, MC2, IQ], BF16, name=f"w2s{s}", tag=f"w2s{s}")
            for s in range(NSLOT)
        ]
        nc.sync.dma_start(out=b1t[:], in_=b1p[:, :, :])
        for s in range(NSLOT):
            nc.gpsimd.dma_start(out=w1s[s][:], in_=w1p[s])
            nc.gpsimd.dma_start(out=w2s[s][:], in_=w2p[s])

        rep_ctx = tc.For_i(0, repeat, 1) if repeat > 1 else None
        if rep_ctx is not None:
            ctx.enter_context(rep_ctx)

        for s, t0, ln in _token_tiles(caps):
            xt = xpool.tile([128, KC1, ln], BF16, tag="xt")
            nc.sync.dma_start(out=xt[:], in_=xT[:, :, t0 : t0 + ln])

            ht = hpool.tile([128, KC2, ln], BF16, tag="ht")
            for mc in range(MC1):
                ps = ps1.tile([128, ln], F32, tag="ps1")
                for kc in range(KC1):
                    nc.tensor.matmul(
                        ps[:],
                        lhsT=w1s[s][:, mc, kc * 128 : (kc + 1) * 128],
                        rhs=xt[:, kc, :],
                        start=(kc == 0),
                        stop=(kc == KC1 - 1),
                    )
                nc.scalar.activation(
                    out=ht[:, mc, :],
                    in_=ps[:],
                    func=gelu,
                    bias=b1t[:, s * MC1 + mc : s * MC1 + mc + 1],
                    scale=1.0,
                )

            yt = ypool.tile([128, MC2, ln], F32, tag="yt")
            for mc in range(MC2):
                ps = ps2.tile([128, ln], F32, tag="ps2")
                for ic in range(KC2):
                    nc.tensor.matmul(
                        ps[:],
                        lhsT=w2s[s][:, mc, ic * 128 : (ic + 1) * 128],
                        rhs=ht[:, ic, :],
                        start=(ic == 0),
                        stop=(ic == KC2 - 1),
                    )
                nc.vector.tensor_copy(out=yt[:, mc, :], in_=ps[:])
            if dummy_out:
                nc.sync.dma_start(out=yT[:, :, :ln], in_=yt[:])
            else:
                nc.sync.dma_start(out=yT[:, :, t0 : t0 + ln], in_=yt[:])

    nc.compile()
    return nc


def _route(x_flat, gate_w, gate_b, alpha):
    """Host gate: returns per-expert (row_indices, combine_scores)."""
    logits = x_flat @ gate_w + gate_b
    m = logits.max(axis=-1, keepdims=True)
    p = np.exp(logits - m)
    p /= p.sum(axis=-1, keepdims=True)
    idx = np.argpartition(p, E - TOP_K, axis=-1)[:, -TOP_K:]  # top-2 (unordered)
    vals = np.take_along_axis(p, idx, axis=-1)
    sc = vals / vals.sum(axis=-1, keepdims=True)
    sc = sc * alpha[idx]
    routes = []
    for e in range(E):
        mask = idx == e  # at most one True per row (top-k distinct)
        rows = np.nonzero(mask.any(axis=1))[0]
        scores = sc[mask]  # row-major order matches `rows`
        routes.append((rows, scores.astype(np.float32)))
    return routes


def _plan(counts):
    """Place 8 experts x SPLIT shards on the 8-core x NSLOT-slot grid.

    Slot s holds experts order[2s] (its SPLIT shards on cores 0..SPLIT-1)
    and order[2s+1] (on cores SPLIT..2*SPLIT-1); cap_s = max of the two
    counts. Sorted placement minimizes sum_s cap_s.
    """
    order = np.argsort(-np.asarray(counts), kind="stable")
    assign = [[None] * NSLOT for _ in range(N_CORES)]
    caps = []
    for s in range(NSLOT):
        ea, eb = int(order[2 * s]), int(order[2 * s + 1])
        for q in range(SPLIT):
            assign[q][s] = (ea, q)
            assign[SPLIT + q][s] = (eb, q)
        caps.append(int(max(counts[ea], counts[eb])))
    return assign, caps


def prepare(hidden_states, gate_w, gate_b, w1, b1, w2, b2, alpha):
    """Host routing + input prep. Returns (nc, in_maps, state)."""
    x = np.asarray(hidden_states, dtype=np.float32)
    gate_w = np.asarray(gate_w, dtype=np.float32)
    gate_b = np.asarray(gate_b, dtype=np.float32)
    w1 = np.asarray(w1, dtype=np.float32)
    b1 = np.asarray(b1, dtype=np.float32)
    w2 = np.asarray(w2, dtype=np.float32)
    b2 = np.asarray(b2, dtype=np.float32)
    alpha = np.asarray(alpha, dtype=np.float32)

    B, S, Hd = x.shape
    T = B * S
    xf = x.reshape(T, Hd)

    routes = _route(xf, gate_w, gate_b, alpha)
    counts = [len(r) for r, _ in routes]
    assign, caps = _plan(counts)
    C = sum(caps)
    offs = np.concatenate([[0], np.cumsum(caps)]).astype(int)

    nc = _build_nc(caps)

    # Per-expert packed tokens [128, KC1, cnt] bf16, shared by the SPLIT
    # cores that hold the expert's shards.
    xTe = {}
    for e in range(E):
        rows, _ = routes[e]
        xTe[e] = np.ascontiguousarray(
            xf[rows].astype(BF16_NP).T.reshape(KC1, 128, len(rows))
            .transpose(1, 0, 2)
        )

    def pack_w(wq):
        # [128k, kc-or-ic chunks * 128 m] from [K, M]: out[p, mc, kc*128+m]
        K, M = wq.shape
        return np.ascontiguousarray(
            wq.reshape(K // 128, 128, M // 128, 128)
            .transpose(1, 2, 0, 3)
            .reshape(128, M // 128, K)
            .astype(BF16_NP)
        )

    in_maps = []
    for c in range(N_CORES):
        xTc = np.zeros((128, KC1, C), dtype=BF16_NP)
        w1c = np.zeros((NSLOT, 128, MC1, H), dtype=BF16_NP)
        w2c = np.zeros((NSLOT, 128, MC2, IQ), dtype=BF16_NP)
        b1c = np.zeros((NSLOT, 128, MC1), dtype=np.float32)
        for s in range(NSLOT):
            e, q = assign[c][s]
            cnt = counts[e]
            xTc[:, :, offs[s] : offs[s] + cnt] = xTe[e]
            w1c[s] = pack_w(w1[e][:, q * IQ : (q + 1) * IQ])
            w2c[s] = pack_w(w2[e][q * IQ : (q + 1) * IQ, :])
            b1c[s] = b1[e][q * IQ : (q + 1) * IQ].reshape(MC1, 128).T
        in_maps.append({"xT": xTc, "w1p": w1c, "w2p": w2c, "b1p": b1c})

    state = dict(
        routes=routes, counts=counts, assign=assign, caps=caps, offs=offs,
        C=C, b2=b2, B=B, S=S, Hd=Hd, T=T,
    )
    return nc, in_maps, state


def finalize(results, state):
    routes, counts = state["routes"], state["counts"]
    assign, offs = state["assign"], state["offs"]
    b2 = state["b2"]
    T, Hd = state["T"], state["Hd"]
    C = state["C"]

    # Sum the SPLIT I-shard partials per expert: [Hd, cnt] each.
    ysum = {}
    for c in range(N_CORES):
        yTc = results[c]["yT"].transpose(1, 0, 2).reshape(Hd, C)
        for s in range(NSLOT):
            e, _q = assign[c][s]
            part = yTc[:, offs[s] : offs[s] + counts[e]]
            ysum[e] = part if e not in ysum else ysum[e] + part

    out = np.zeros((T, Hd), dtype=np.float32)
    for e in range(E):
        rows, scores = routes[e]
        if not len(rows):
            continue
        out[rows] += scores[:, None] * (ysum[e].T + b2[e])
    return out.reshape(state["B"], state["S"], Hd)


def kernel(hidden_states, gate_w, gate_b, w1, b1, w2, b2, alpha):
    nc, in_maps, state = prepare(
        hidden_states, gate_w, gate_b, w1, b1, w2, b2, alpha
    )
    res = run_bass_kernel_spmd(nc, in_maps, list(range(N_CORES)))
    return finalize(res.results, state)


# revision 14
# speedup vs baseline: 1.0250x; 1.0091x over previous
"""MoE top-2 routing kernel for 8 Trainium2 NeuronCores.

Strategy (expert-parallel with I-sharding, per spec sharding hint):
  - Host computes the (cheap) gate: softmax -> top-2 -> renormalized scores.
  - Each expert's MLP is split into SPLIT=4 shards along the intermediate
    dim I; the 8 experts x 4 shards = 32 shard-tasks are placed on an
    8-core x 4-slot grid (2 experts per slot column, paired big/small by
    routed token count) so per-core work is near the 874us PE ideal.
  - Weights are bf16 and fully SBUF-resident (128 KiB/partition for both
    layers), so tokens stream while weights load once: DMA drops from
    ~320 MB/core (fp32r re-streaming baseline) to ~120 MB/core and the
    kernel is PE-bound at ~1 cycle/row bf16.
  - Host sums the 4 I-shard partials per expert, applies combine scores
    + b2, and scatter-adds into the full output.

Per-core device work (C ~= 16.4k shard-token columns, 128 PE cycles each):
  fc1 quarter: h = gelu(x @ w1q + b1q)   8x8 mm chunks per 512-col tile
  fc2 quarter: y_partial = h @ w2q       8x8 mm chunks per 512-col tile
  PE floor ~= C * 128 cycles; with all 8 cores busy the chip power-throttles
  the PE to ~2.0 GHz (measured 254-263 ns per 512-row mm vs 213 ideal;
  1 core alone measures 219), so the envelope is ~1.05-1.1 ms/core.
  Measured 1.10 ms -> ~99% of the 8-core-busy PE envelope; DMA ~= 120 MB
  fully overlapped. fp8 DoubleRow (2x PE) is blocked by accuracy: e4m3
  rounding ~5% rms vs the 2% tolerance.
"""

import sys

sys.path.insert(0, "/opt/trn_rl_repo")

from contextlib import ExitStack

import numpy as np
import ml_dtypes

from concourse import bacc, mybir, tile
from concourse.bass_utils import run_bass_kernel_spmd

E, H, I = 8, 1024, 4096
TOP_K = 2
N_CORES = 8

SPLIT = 4          # I-shards per expert
NSLOT = 4          # shard slots per core (E * SPLIT / N_CORES)
IQ = I // SPLIT    # 1024 intermediate dims per shard
MC1 = IQ // 128    # fc1 output chunks per shard
KC1 = H // 128     # fc1 contraction chunks
MC2 = H // 128     # fc2 output chunks
KC2 = IQ // 128    # fc2 contraction chunks per shard

TT = 512           # token tile (one PSUM bank of fp32)

F32 = mybir.dt.float32
BF16 = mybir.dt.bfloat16
BF16_NP = ml_dtypes.bfloat16


def _token_tiles(caps):
    """Static schedule: (slot, col_offset, len) covering sum(caps) columns."""
    out = []
    off = 0
    for s, cap in enumerate(caps):
        o = 0
        while o < cap:
            ln = min(TT, cap - o)
            out.append((s, off + o, ln))
            o += ln
        off += cap
    return out


def _build_nc(caps, repeat=1, dummy_out=False):
    """One SPMD program: NSLOT resident expert-shard MLPs over token columns.

    dummy_out=True keeps the full compute + DMA volume but writes every
    tile's y to one small DRAM region, so timing runs don't churn a 67MB
    donated output buffer between calls (the axon tunnel time is bimodal
    under large-buffer churn, which poisons the repeat-delta estimate).
    """
    C = sum(caps)
    nc = bacc.Bacc(
        "TRN2", target_bir_lowering=False, debug=False, num_devices=N_CORES
    )
    xT = nc.dram_tensor("xT", [128, KC1, C], BF16, kind="ExternalInput").ap()
    w1p = nc.dram_tensor(
        "w1p", [NSLOT, 128, MC1, H], BF16, kind="ExternalInput"
    ).ap()
    w2p = nc.dram_tensor(
        "w2p", [NSLOT, 128, MC2, IQ], BF16, kind="ExternalInput"
    ).ap()
    b1p = nc.dram_tensor("b1p", [NSLOT, 128, MC1], F32, kind="ExternalInput").ap()
    yC = TT if dummy_out else C
    yT = nc.dram_tensor("yT", [128, MC2, yC], F32, kind="ExternalOutput").ap()

    gelu = mybir.ActivationFunctionType.Gelu

    with tile.TileContext(nc) as tc, ExitStack() as ctx:
        wpool = ctx.enter_context(tc.tile_pool(name="w", bufs=1))
        xpool = ctx.enter_context(tc.tile_pool(name="x", bufs=2))
        hpool = ctx.enter_context(tc.tile_pool(name="h", bufs=2))
        ypool = ctx.enter_context(tc.tile_pool(name="y", bufs=1))
        ps1 = ctx.enter_context(tc.tile_pool(name="ps1", bufs=3, space="PSUM"))
        ps2 = ctx.enter_context(tc.tile_pool(name="ps2", bufs=3, space="PSUM"))

        b1t = wpool.tile([128, NSLOT * MC1], F32)
        # Per-slot weight tiles: Tile tracks dependencies per tile, so slot
        # 0's first matmul waits only on slot 0's 2MB DMA (~7us) instead of
        # the whole 16MB preload (~45us single-launch startup bubble).
        w1s = [
            wpool.tile([128, MC1, H], BF16, name=f"w1s{s}", tag=f"w1s{s}")
            for s in range(NSLOT)
        ]
        w2s = [
            wpool.tile([128, MC2, IQ], BF16, name=f"w2s{s}", tag=f"w2s{s}")
            for s in range(NSLOT)
        ]
        nc.sync.dma_start(out=b1t[:], in_=b1p[:, :, :])
        for s in range(NSLOT):
            nc.gpsimd.dma_start(out=w1s[s][:], in_=w1p[s])
            nc.gpsimd.dma_start(out=w2s[s][:], in_=w2p[s])

        rep_ctx = tc.For_i(0, repeat, 1) if repeat > 1 else None
        if rep_ctx is not None:
            ctx.enter_context(rep_ctx)

        for s, t0, ln in _token_tiles(caps):
            xt = xpool.tile([128, KC1, ln], BF16, tag="xt")
            nc.sync.dma_start(out=xt[:], in_=xT[:, :, t0 : t0 + ln])

            HH = KC2 // 2
            hta = hpool.tile([128, HH, ln], BF16, tag="hta")
            htb = hpool.tile([128, KC2 - HH, ln], BF16, tag="htb")
            for mc in range(MC1):
                ps = ps1.tile([128, ln], F32, tag="ps1")
                for kc in range(KC1):
                    nc.tensor.matmul(
                        ps[:],
                        lhsT=w1s[s][:, mc, kc * 128 : (kc + 1) * 128],
                        rhs=xt[:, kc, :],
                        start=(kc == 0),
                        stop=(kc == KC1 - 1),
                    )
                nc.scalar.activation(
                    out=(hta[:, mc, :] if mc < HH else htb[:, mc - HH, :]),
                    in_=ps[:],
                    func=gelu,
                    bias=b1t[:, s * MC1 + mc : s * MC1 + mc + 1],
                    scale=1.0,
                )

            yt = ypool.tile([128, MC2, ln], F32, tag="yt")
            for mc in range(MC2):
                ps = ps2.tile([128, ln], F32, tag="ps2")
                for ic in range(KC2):
                    nc.tensor.matmul(
                        ps[:],
                        lhsT=w2s[s][:, mc, ic * 128 : (ic + 1) * 128],
                        rhs=(hta[:, ic, :] if ic < HH else htb[:, ic - HH, :]),
                        start=(ic == 0),
                        stop=(ic == KC2 - 1),
                    )
                nc.vector.tensor_copy(out=yt[:, mc, :], in_=ps[:])
            if dummy_out:
                nc.sync.dma_start(out=yT[:, :, :ln], in_=yt[:])
            else:
                nc.sync.dma_start(out=yT[:, :, t0 : t0 + ln], in_=yt[:])

    nc.compile()
    return nc


def _route(x_flat, gate_w, gate_b, alpha):
    """Host gate: returns per-expert (row_indices, combine_scores)."""
    logits = x_flat @ gate_w + gate_b
    m = logits.max(axis=-1, keepdims=True)
    p = np.exp(logits - m)
    p /= p.sum(axis=-1, keepdims=True)
    idx = np.argpartition(p, E - TOP_K, axis=-1)[:, -TOP_K:]  # top-2 (unordered)
    vals = np.take_along_axis(p, idx, axis=-1)
    sc = vals / vals.sum(axis=-1, keepdims=True)
    sc = sc * alpha[idx]
    routes = []
    for e in range(E):
        mask = idx == e  # at most one True per row (top-k distinct)
        rows = np.nonzero(mask.any(axis=1))[0]
        scores = sc[mask]  # row-major order matches `rows`
        routes.append((rows, scores.astype(np.float32)))
    return routes


def _plan(counts):
    """Place 8 experts x SPLIT shards on the 8-core x NSLOT-slot grid.

    Slot s holds experts order[2s] (its SPLIT shards on cores 0..SPLIT-1)
    and order[2s+1] (on cores SPLIT..2*SPLIT-1); cap_s = max of the two
    counts. Sorted placement minimizes sum_s cap_s.
    """
    order = np.argsort(-np.asarray(counts), kind="stable")
    assign = [[None] * NSLOT for _ in range(N_CORES)]
    caps = []
    for s in range(NSLOT):
        ea, eb = int(order[2 * s]), int(order[2 * s + 1])
        for q in range(SPLIT):
            assign[q][s] = (ea, q)
            assign[SPLIT + q][s] = (eb, q)
        caps.append(int(max(counts[ea], counts[eb])))
    return assign, caps


def prepare(hidden_states, gate_w, gate_b, w1, b1, w2, b2, alpha):
    """Host routing + input prep. Returns (nc, in_maps, state)."""
    x = np.asarray(hidden_states, dtype=np.float32)
    gate_w = np.asarray(gate_w, dtype=np.float32)
    gate_b = np.asarray(gate_b, dtype=np.float32)
    w1 = np.asarray(w1, dtype=np.float32)
    b1 = np.asarray(b1, dtype=np.float32)
    w2 = np.asarray(w2, dtype=np.float32)
    b2 = np.asarray(b2, dtype=np.float32)
    alpha = np.asarray(alpha, dtype=np.float32)

    B, S, Hd = x.shape
    T = B * S
    xf = x.reshape(T, Hd)

    routes = _route(xf, gate_w, gate_b, alpha)
    counts = [len(r) for r, _ in routes]
    assign, caps = _plan(counts)
    C = sum(caps)
    offs = np.concatenate([[0], np.cumsum(caps)]).astype(int)

    nc = _build_nc(caps)

    # Per-expert packed tokens [128, KC1, cnt] bf16, shared by the SPLIT
    # cores that hold the expert's shards.
    xTe = {}
    for e in range(E):
        rows, _ = routes[e]
        xTe[e] = np.ascontiguousarray(
            xf[rows].astype(BF16_NP).T.reshape(KC1, 128, len(rows))
            .transpose(1, 0, 2)
        )

    def pack_w(wq):
        # [128k, kc-or-ic chunks * 128 m] from [K, M]: out[p, mc, kc*128+m]
        K, M = wq.shape
        return np.ascontiguousarray(
            wq.reshape(K // 128, 128, M // 128, 128)
            .transpose(1, 2, 0, 3)
            .reshape(128, M // 128, K)
            .astype(BF16_NP)
        )

    in_maps = []
    for c in range(N_CORES):
        xTc = np.zeros((128, KC1, C), dtype=BF16_NP)
        w1c = np.zeros((NSLOT, 128, MC1, H), dtype=BF16_NP)
        w2c = np.zeros((NSLOT, 128, MC2, IQ), dtype=BF16_NP)
        b1c = np.zeros((NSLOT, 128, MC1), dtype=np.float32)
        for s in range(NSLOT):
            e, q = assign[c][s]
            cnt = counts[e]
            xTc[:, :, offs[s] : offs[s] + cnt] = xTe[e]
            w1c[s] = pack_w(w1[e][:, q * IQ : (q + 1) * IQ])
            w2c[s] = pack_w(w2[e][q * IQ : (q + 1) * IQ, :])
            b1c[s] = b1[e][q * IQ : (q + 1) * IQ].reshape(MC1, 128).T
        in_maps.append({"xT": xTc, "w1p": w1c, "w2p": w2c, "b1p": b1c})

    state = dict(
        routes=routes, counts=counts, assign=assign, caps=caps, offs=offs,
        C=C, b2=b2, B=B, S=S, Hd=Hd, T=T,
    )
    return nc, in_maps, state


def finalize(results, state):
    routes, counts = state["routes"], state["counts"]
    assign, offs = state["assign"], state["offs"]
    b2 = state["b2"]
    T, Hd = state["T"], state["Hd"]
    C = state["C"]

    # Sum the SPLIT I-shard partials per expert: [Hd, cnt] each.
    ysum = {}
    for c in range(N_CORES):
        yTc = results[c]["yT"].transpose(1, 0, 2).reshape(Hd, C)
        for s in range(NSLOT):
            e, _q = assign[c][s]
            part = yTc[:, offs[s] : offs[s] + counts[e]]
            ysum[e] = part if e not in ysum else ysum[e] + part

    out = np.zeros((T, Hd), dtype=np.float32)
    for e in range(E):
        rows, scores = routes[e]
        if not len(rows):
            continue
        out[rows] += scores[:, None] * (ysum[e].T + b2[e])
    return out.reshape(state["B"], state["S"], Hd)


def kernel(hidden_states, gate_w, gate_b, w1, b1, w2, b2, alpha):
    nc, in_maps, state = prepare(
        hidden_states, gate_w, gate_b, w1, b1, w2, b2, alpha
    )
    res = run_bass_kernel_spmd(nc, in_maps, list(range(N_CORES)))
    return finalize(res.results, state)


# revision 16
# speedup vs baseline: 1.0347x; 1.0095x over previous
"""MoE top-2 routing kernel for 8 Trainium2 NeuronCores.

Strategy (expert-parallel with I-sharding, per spec sharding hint):
  - Host computes the (cheap) gate: softmax -> top-2 -> renormalized scores.
  - Each expert's MLP is split into SPLIT=4 shards along the intermediate
    dim I; the 8 experts x 4 shards = 32 shard-tasks are placed on an
    8-core x 4-slot grid (2 experts per slot column, paired big/small by
    routed token count) so per-core work is near the 874us PE ideal.
  - Weights are bf16 and fully SBUF-resident (128 KiB/partition for both
    layers), so tokens stream while weights load once: DMA drops from
    ~320 MB/core (fp32r re-streaming baseline) to ~120 MB/core and the
    kernel is PE-bound at ~1 cycle/row bf16.
  - Host sums the 4 I-shard partials per expert, applies combine scores
    + b2, and scatter-adds into the full output.

Per-core device work (C ~= 16.4k shard-token columns, 128 PE cycles each):
  fc1 quarter: h = gelu(x @ w1q + b1q)   8x8 mm chunks per 512-col tile
  fc2 quarter: y_partial = h @ w2q       8x8 mm chunks per 512-col tile
  PE floor ~= C * 128 cycles; with all 8 cores busy the chip power-throttles
  the PE to ~2.0 GHz (measured 254-263 ns per 512-row mm vs 213 ideal;
  1 core alone measures 219), so the envelope is ~1.05-1.1 ms/core.
  Measured 1.10 ms -> ~99% of the 8-core-busy PE envelope; DMA ~= 120 MB
  fully overlapped. fp8 DoubleRow (2x PE) is blocked by accuracy: e4m3
  rounding ~5% rms vs the 2% tolerance.
"""

import sys

sys.path.insert(0, "/opt/trn_rl_repo")

from contextlib import ExitStack

import numpy as np
import ml_dtypes

from concourse import bacc, mybir, tile
from concourse.bass_utils import run_bass_kernel_spmd

E, H, I = 8, 1024, 4096
TOP_K = 2
N_CORES = 8

SPLIT = 4          # I-shards per expert
NSLOT = 4          # shard slots per core (E * SPLIT / N_CORES)
IQ = I // SPLIT    # 1024 intermediate dims per shard
MC1 = IQ // 128    # fc1 output chunks per shard
KC1 = H // 128     # fc1 contraction chunks
MC2 = H // 128     # fc2 output chunks
KC2 = IQ // 128    # fc2 contraction chunks per shard

TT = 512           # token tile (one PSUM bank of fp32)

F32 = mybir.dt.float32
BF16 = mybir.dt.bfloat16
BF16_NP = ml_dtypes.bfloat16


def _token_tiles(caps):
    """Static schedule: (slot, col_offset, len) covering sum(caps) columns."""
    out = []
    off = 0
    for s, cap in enumerate(caps):
        o = 0
        while o < cap:
            ln = min(TT, cap - o)
            out.append((s, off + o, ln))
            o += ln
        off += cap
    return out


def _build_nc(caps, repeat=1, dummy_out=False):
    """One SPMD program: NSLOT resident expert-shard MLPs over token columns.

    dummy_out=True keeps the full compute + DMA volume but writes every
    tile's y to one small DRAM region, so timing runs don't churn a 67MB
    donated output buffer between calls (the axon tunnel time is bimodal
    under large-buffer churn, which poisons the repeat-delta estimate).
    """
    C = sum(caps)
    nc = bacc.Bacc(
        "TRN2", target_bir_lowering=False, debug=False, num_devices=N_CORES
    )
    xT = nc.dram_tensor("xT", [128, KC1, C], BF16, kind="ExternalInput").ap()
    w1p = nc.dram_tensor(
        "w1p", [NSLOT, 128, MC1, H], BF16, kind="ExternalInput"
    ).ap()
    w2p = nc.dram_tensor(
        "w2p", [NSLOT, 128, MC2, IQ], BF16, kind="ExternalInput"
    ).ap()
    b1p = nc.dram_tensor("b1p", [NSLOT, 128, MC1], F32, kind="ExternalInput").ap()
    yC = TT if dummy_out else C
    yT = nc.dram_tensor("yT", [128, MC2, yC], F32, kind="ExternalOutput").ap()

    gelu = mybir.ActivationFunctionType.Gelu

    with tile.TileContext(nc) as tc, ExitStack() as ctx:
        wpool = ctx.enter_context(tc.tile_pool(name="w", bufs=1))
        xpool = ctx.enter_context(tc.tile_pool(name="x", bufs=2))
        hpool = ctx.enter_context(tc.tile_pool(name="h", bufs=2))
        ypool = ctx.enter_context(tc.tile_pool(name="y", bufs=1))
        ps1 = ctx.enter_context(tc.tile_pool(name="ps1", bufs=3, space="PSUM"))
        ps2 = ctx.enter_context(tc.tile_pool(name="ps2", bufs=3, space="PSUM"))

        b1t = wpool.tile([128, NSLOT * MC1], F32)
        # Per-slot weight tiles: Tile tracks dependencies per tile, so slot
        # 0's first matmul waits only on slot 0's w1 DMA instead of the
        # whole 16MB preload (~45us single-launch startup bubble). Slot 0's
        # w1 is further split in two tiles so the first fc1 group starts
        # after 1MB (~4us, matching the concurrent 1MB x-tile DMA).
        w1s0 = [
            wpool.tile([128, MC1 // 2, H], BF16, name=f"w1s0{i}", tag=f"w1s0{i}")
            for i in range(2)
        ]
        w1s = [None] + [
            wpool.tile([128, MC1, H], BF16, name=f"w1s{s}", tag=f"w1s{s}")
            for s in range(1, NSLOT)
        ]
        w2s = [
            wpool.tile([128, MC2, IQ], BF16, name=f"w2s{s}", tag=f"w2s{s}")
            for s in range(NSLOT)
        ]

        def lhs1(s, mc):
            if s == 0:
                return w1s0[mc // (MC1 // 2)][:, mc % (MC1 // 2), :]
            return w1s[s][:, mc, :]

        nc.sync.dma_start(out=b1t[:], in_=b1p[:, :, :])
        for i in range(2):
            nc.gpsimd.dma_start(
                out=w1s0[i][:],
                in_=w1p[0, :, i * (MC1 // 2) : (i + 1) * (MC1 // 2), :],
            )
        nc.gpsimd.dma_start(out=w2s[0][:], in_=w2p[0])
        for s in range(1, NSLOT):
            nc.gpsimd.dma_start(out=w1s[s][:], in_=w1p[s])
            nc.gpsimd.dma_start(out=w2s[s][:], in_=w2p[s])

        rep_ctx = tc.For_i(0, repeat, 1) if repeat > 1 else None
        if rep_ctx is not None:
            ctx.enter_context(rep_ctx)

        for s, t0, ln in _token_tiles(caps):
            xt = xpool.tile([128, KC1, ln], BF16, tag="xt")
            nc.sync.dma_start(out=xt[:], in_=xT[:, :, t0 : t0 + ln])

            HH = KC2 // 2
            hta = hpool.tile([128, HH, ln], BF16, tag="hta")
            htb = hpool.tile([128, KC2 - HH, ln], BF16, tag="htb")
            for mc in range(MC1):
                ps = ps1.tile([128, ln], F32, tag="ps1")
                lw = lhs1(s, mc)
                for kc in range(KC1):
                    nc.tensor.matmul(
                        ps[:],
                        lhsT=lw[:, kc * 128 : (kc + 1) * 128],
                        rhs=xt[:, kc, :],
                        start=(kc == 0),
                        stop=(kc == KC1 - 1),
                    )
                nc.scalar.activation(
                    out=(hta[:, mc, :] if mc < HH else htb[:, mc - HH, :]),
                    in_=ps[:],
                    func=gelu,
                    bias=b1t[:, s * MC1 + mc : s * MC1 + mc + 1],
                    scale=1.0,
                )

            yt = ypool.tile([128, MC2, ln], F32, tag="yt")
            for mc in range(MC2):
                ps = ps2.tile([128, ln], F32, tag="ps2")
                for ic in range(KC2):
                    nc.tensor.matmul(
                        ps[:],
                        lhsT=w2s[s][:, mc, ic * 128 : (ic + 1) * 128],
                        rhs=(hta[:, ic, :] if ic < HH else htb[:, ic - HH, :]),
                        start=(ic == 0),
                        stop=(ic == KC2 - 1),
                    )
                nc.vector.tensor_copy(out=yt[:, mc, :], in_=ps[:])
            if dummy_out:
                nc.sync.dma_start(out=yT[:, :, :ln], in_=yt[:])
            else:
                nc.sync.dma_start(out=yT[:, :, t0 : t0 + ln], in_=yt[:])

    nc.compile()
    return nc


def _route(x_flat, gate_w, gate_b, alpha):
    """Host gate: returns per-expert (row_indices, combine_scores)."""
    logits = x_flat @ gate_w + gate_b
    m = logits.max(axis=-1, keepdims=True)
    p = np.exp(logits - m)
    p /= p.sum(axis=-1, keepdims=True)
    idx = np.argpartition(p, E - TOP_K, axis=-1)[:, -TOP_K:]  # top-2 (unordered)
    vals = np.take_along_axis(p, idx, axis=-1)
    sc = vals / vals.sum(axis=-1, keepdims=True)
    sc = sc * alpha[idx]
    routes = []
    for e in range(E):
        mask = idx == e  # at most one True per row (top-k distinct)
        rows = np.nonzero(mask.any(axis=1))[0]
        scores = sc[mask]  # row-major order matches `rows`
        routes.append((rows, scores.astype(np.float32)))
    return routes


def _plan(counts):
    """Place 8 experts x SPLIT shards on the 8-core x NSLOT-slot grid.

    Slot s holds experts order[2s] (its SPLIT shards on cores 0..SPLIT-1)
    and order[2s+1] (on cores SPLIT..2*SPLIT-1); cap_s = max of the two
    counts. Sorted placement minimizes sum_s cap_s.
    """
    order = np.argsort(-np.asarray(counts), kind="stable")
    assign = [[None] * NSLOT for _ in range(N_CORES)]
    caps = []
    for s in range(NSLOT):
        ea, eb = int(order[2 * s]), int(order[2 * s + 1])
        for q in range(SPLIT):
            assign[q][s] = (ea, q)
            assign[SPLIT + q][s] = (eb, q)
        caps.append(int(max(counts[ea], counts[eb])))
    return assign, caps


def prepare(hidden_states, gate_w, gate_b, w1, b1, w2, b2, alpha):
    """Host routing + input prep. Returns (nc, in_maps, state)."""
    x = np.asarray(hidden_states, dtype=np.float32)
    gate_w = np.asarray(gate_w, dtype=np.float32)
    gate_b = np.asarray(gate_b, dtype=np.float32)
    w1 = np.asarray(w1, dtype=np.float32)
    b1 = np.asarray(b1, dtype=np.float32)
    w2 = np.asarray(w2, dtype=np.float32)
    b2 = np.asarray(b2, dtype=np.float32)
    alpha = np.asarray(alpha, dtype=np.float32)

    B, S, Hd = x.shape
    T = B * S
    xf = x.reshape(T, Hd)

    routes = _route(xf, gate_w, gate_b, alpha)
    counts = [len(r) for r, _ in routes]
    assign, caps = _plan(counts)
    C = sum(caps)
    offs = np.concatenate([[0], np.cumsum(caps)]).astype(int)

    nc = _build_nc(caps)

    # Per-expert packed tokens [128, KC1, cnt] bf16, shared by the SPLIT
    # cores that hold the expert's shards.
    xTe = {}
    for e in range(E):
        rows, _ = routes[e]
        xTe[e] = np.ascontiguousarray(
            xf[rows].astype(BF16_NP).T.reshape(KC1, 128, len(rows))
            .transpose(1, 0, 2)
        )

    def pack_w(wq):
        # [128k, kc-or-ic chunks * 128 m] from [K, M]: out[p, mc, kc*128+m]
        K, M = wq.shape
        return np.ascontiguousarray(
            wq.reshape(K // 128, 128, M // 128, 128)
            .transpose(1, 2, 0, 3)
            .reshape(128, M // 128, K)
            .astype(BF16_NP)
        )

    in_maps = []
    for c in range(N_CORES):
        xTc = np.zeros((128, KC1, C), dtype=BF16_NP)
        w1c = np.zeros((NSLOT, 128, MC1, H), dtype=BF16_NP)
        w2c = np.zeros((NSLOT, 128, MC2, IQ), dtype=BF16_NP)
        b1c = np.zeros((NSLOT, 128, MC1), dtype=np.float32)
        for s in range(NSLOT):
            e, q = assign[c][s]
            cnt = counts[e]
            xTc[:, :, offs[s] : offs[s] + cnt] = xTe[e]
            w1c[s] = pack_w(w1[e][:, q * IQ : (q + 1) * IQ])
            w2c[s] = pack_w(w2[e][q * IQ : (q + 1) * IQ, :])
            b1c[s] = b1[e][q * IQ : (q + 1) * IQ].reshape(MC1, 128).T
        in_maps.append({"xT": xTc, "w1p": w1c, "w2p": w2c, "b1p": b1c})

    state = dict(
        routes=routes, counts=counts, assign=assign, caps=caps, offs=offs,
        C=C, b2=b2, B=B, S=S, Hd=Hd, T=T,
    )
    return nc, in_maps, state


def finalize(results, state):
    routes, counts = state["routes"], state["counts"]
    assign, offs = state["assign"], state["offs"]
    b2 = state["b2"]
    T, Hd = state["T"], state["Hd"]
    C = state["C"]

    # Sum the SPLIT I-shard partials per expert: [Hd, cnt] each.
    ysum = {}
    for c in range(N_CORES):
        yTc = results[c]["yT"].transpose(1, 0, 2).reshape(Hd, C)
        for s in range(NSLOT):
            e, _q = assign[c][s]
            part = yTc[:, offs[s] : offs[s] + counts[e]]
            ysum[e] = part if e not in ysum else ysum[e] + part

    out = np.zeros((T, Hd), dtype=np.float32)
    for e in range(E):
        rows, scores = routes[e]
        if not len(rows):
            continue
        out[rows] += scores[:, None] * (ysum[e].T + b2[e])
    return out.reshape(state["B"], state["S"], Hd)


def kernel(hidden_states, gate_w, gate_b, w1, b1, w2, b2, alpha):
    nc, in_maps, state = prepare(
        hidden_states, gate_w, gate_b, w1, b1, w2, b2, alpha
    )
    res = run_bass_kernel_spmd(nc, in_maps, list(range(N_CORES)))
    return finalize(res.results, state)
